# revision 1
# baseline (speedup 1.0000x reference)
"""Trainium2 Bass kernel for nn_AttnReweight (superpixel-reweighted attention).

Math (per batch b, head hd, pixel (h,w), key k in a 7x7 window):
    w[b,h,w,k] = sum_{s in 3x3 superpixel nbhd} Pi[b,h,w,s] * Pj[b,s,h,w,k]
    out = (w * exp(attn)) / (eps + sum_k w * exp(attn))
(The reference's max-shift cancels in the ratio; attn ~ N(0,1) so exp() is
safe in fp32 without it.)

Sharding: 8 cores = 2 batches x 4 row-bands of 64 rows. Each core gets
  - its attn shard, pre-swizzled to the on-chip (tile, head, block, pixel)
    layout so loads/stores are two maximal contiguous DMAs per (tile, head)
  - a "slab" shard: for each of its 70 rows (64 + 3 halo each side, rows
    clamped at the image border) the 5 superpixel-table rows that any query
    window positioned at that row can touch, zero-masked where the plane
    index falls outside the 32x32 superpixel grid.
All remaining work is on-device and identical on every core (SPMD):
per-pixel 5x5 window extraction, per-block (8x8-pixel) region tiles,
the 9-term superpixel einsum, exp/normalize, and the output writeback.
"""

import sys

sys.path.insert(0, "/opt/trn_rl_repo")

import numpy as np

import concourse.bass as bass
import concourse.tile as tile
from concourse import bacc, mybir
from contextlib import ExitStack

F32 = mybir.dt.float32
BF16 = mybir.dt.bfloat16

# problem geometry (hardcoded per the harness contract)
B, HD, H, W, K = 2, 4, 256, 256, 49
SH = SW = 32
N_CORES = 8
BAND = 64          # pixel rows per core
HALO = 3
NROW = BAND + 2 * HALO          # 70 A rows per core
NT = 2                          # tiles per core (block-row halves)
HBT = 4                         # block-rows per tile
NBW = 32                        # block-cols
P = HBT * NBW                   # 128 partitions (blocks) per tile
NQ = 14 * 14                    # region pixels per block
NI = 64                         # pixels per block
NK = 49
NS = 9
APAD = 75                       # 3 pixels * 25 on each w side
AFS = APAD + 256 * 25 + APAD    # A free size (w-major, 25-patch inner)
G25FS = NQ * 25                 # 4900
NQ16 = 14 * 16                  # padded region row pitch
G9FS = NS * NQ16                # 2016
EFS = NI * NK                   # 3136 (compact i,k)
EFSP = NI * 56                  # 3584 (k padded to 56 for alignment)
WC = 32                         # slab w-chunk
SLABPAD = 64
SLABFS = WC * 160 + 2 * SLABPAD


def APx(t, off, dims):
    return bass.AP(t.tensor, off, [list(d) for d in dims])


def build_graph():
    nc = bacc.Bacc("TRN2", target_bir_lowering=False, debug=False,
                   num_devices=N_CORES)
    attn_d = nc.dram_tensor("attn", [NT, HD, P, EFS], F32, kind="ExternalInput").ap()
    slab_d = nc.dram_tensor("slab", [NROW, W, 5, SW], BF16, kind="ExternalInput").ap()
    out_d = nc.dram_tensor("out", [NT, HD, P, EFS], F32, kind="ExternalOutput").ap()

    mult, add = mybir.AluOpType.mult, mybir.AluOpType.add

    with tile.TileContext(nc) as tc, ExitStack() as ctx:
        slab_pool = ctx.enter_context(tc.tile_pool(name="slab", bufs=2))
        a_pool = ctx.enter_context(tc.tile_pool(name="apool", bufs=1))
        g25_pool = ctx.enter_context(tc.tile_pool(name="g25", bufs=2))
        g9_pool = ctx.enter_context(tc.tile_pool(name="g9", bufs=2))
        pix_pool = ctx.enter_context(tc.tile_pool(name="pix", bufs=2))
        e_pool = ctx.enter_context(tc.tile_pool(name="epool", bufs=2))
        eb_pool = ctx.enter_context(tc.tile_pool(name="ebpool", bufs=2))
        y_pool = ctx.enter_context(tc.tile_pool(name="ypool", bufs=3))
        w_pool = ctx.enter_context(tc.tile_pool(name="wpool", bufs=2))
        tmp_pool = ctx.enter_context(tc.tile_pool(name="tmp", bufs=3))
        wg_pool = ctx.enter_context(tc.tile_pool(name="wgpool", bufs=1))
        s_pool = ctx.enter_context(tc.tile_pool(name="spool", bufs=4))
        d_pool = ctx.enter_context(tc.tile_pool(name="dstage", bufs=1, space="DRAM"))

        A = a_pool.tile([NROW, AFS], BF16)
        Ad = d_pool.tile([NROW, AFS], BF16)
        # zero the w-padding columns once (read by the full-width G25 DMA)
        nc.vector.memset(APx(A, 0, [[AFS, NROW], [1, APAD]]), 0.0)
        nc.vector.memset(APx(A, APAD + 256 * 25, [[AFS, NROW], [1, APAD]]), 0.0)

        # ---- stage 1: slab load + per-pixel 5x5 window extraction into A
        # A[r, 75 + w*25 + th*5 + tw] = slab[r, w, th, (w//8) + tw - 2]
        for c in range(W // WC):
            SB = slab_pool.tile([NROW, SLABFS], BF16)
            nc.vector.memset(APx(SB, 0, [[SLABFS, NROW], [1, SLABPAD]]), 0.0)
            nc.vector.memset(
                APx(SB, SLABPAD + WC * 160, [[SLABFS, NROW], [1, SLABPAD]]), 0.0)
            nc.sync.dma_start(
                APx(SB, SLABPAD, [[SLABFS, NROW], [1, WC * 160]]),
                APx(slab_d, c * WC * 160, [[W * 160, NROW], [1, WC * 160]]),
            )
            nwb = WC // 8
            src = APx(SB, SLABPAD + (c * nwb) - 2,
                      [[SLABFS, NROW], [8 * 160 + 1, nwb], [160, 8], [32, 5], [1, 5]])
            dst = APx(A, APAD + c * WC * 25,
                      [[AFS, NROW], [200, nwb], [25, 8], [5, 5], [1, 5]])
            nc.vector.tensor_copy(dst, src)

        # zero window columns whose superpixel column falls outside [0,32)
        for w0, nw, tc0, ntc in ((0, 8, 0, 2), (8, 8, 0, 1),
                                 (240, 8, 4, 1), (248, 8, 3, 2)):
            nc.vector.memset(
                APx(A, APAD + w0 * 25 + tc0,
                    [[AFS, NROW], [25, nw], [5, 5], [1, ntc]]), 0.0)
        # fill the w-padding with the border pixel's patch, re-expressed in
        # the out-of-range region position's frame (clipped key pixels)
        nc.vector.tensor_copy(
            APx(A, 0 * 25 + 2, [[AFS, NROW], [25, 3], [5, 5], [1, 3]]),
            APx(A, APAD + 0 * 25 + 1, [[AFS, NROW], [0, 3], [5, 5], [1, 3]]),
        )
        nc.vector.tensor_copy(
            APx(A, APAD + 256 * 25 + 0, [[AFS, NROW], [25, 3], [5, 5], [1, 3]]),
            APx(A, APAD + 255 * 25 + 1, [[AFS, NROW], [0, 3], [5, 5], [1, 3]]),
        )
        # stage A to DRAM (SBUF APs cannot express the partition-crossing
        # A -> G25 rearrange on both sides; DRAM APs are flat)
        nc.sync.dma_start(Ad[:], A[:])

        # ---- per-tile processing
        for T in range(NT):
            # G25[p = hbl*32+wb, (qh*14+qw)*25 + t] = A[32T+8hbl+qh, w=8wb+qw-3, t]
            G25 = g25_pool.tile([P, G25FS], BF16)
            for hbl in range(HBT):
                nc.sync.dma_start(
                    APx(G25, hbl * 32 * G25FS,
                        [[G25FS, NBW], [14 * 25, 14], [1, 350]]),
                    APx(Ad, (32 * T + 8 * hbl) * AFS + APAD - 3 * 25,
                        [[200, NBW], [AFS, 14], [1, 350]]),
                )

            # ---- G9: rectangularize per (s, dd); ACT + GpSimd do the copies
            G9 = g9_pool.tile([P, G9FS], BF16)
            nc.gpsimd.memset(
                APx(G9, 14, [[G9FS, P], [16, NS * 14], [1, 2]]), 0.0)
            engs = [nc.scalar, nc.gpsimd]
            ci = 0
            for si in range(NS):
                dh, dw = si // 3 - 1, si % 3 - 1
                for ddh in (-1, 0, 1):
                    for ddw in (-1, 0, 1):
                        qh0, nqh = {(-1): (0, 3), 0: (3, 8), 1: (11, 3)}[ddh]
                        qw0, nqw = {(-1): (0, 3), 0: (3, 8), 1: (11, 3)}[ddw]
                        tcol = (dh - ddh + 2) * 5 + (dw - ddw + 2)
                        src = APx(G25, (qh0 * 14 + qw0) * 25 + tcol,
                                  [[G25FS, P], [14 * 25, nqh], [25, nqw]])
                        dst = APx(G9, si * NQ16 + qh0 * 16 + qw0,
                                  [[G9FS, P], [16, nqh], [1, nqw]])
                        eng = engs[ci % 2]
                        ci += 1
                        if eng is nc.scalar:
                            eng.copy(dst, src)
                        else:
                            eng.tensor_copy(dst, src)

            # ---- einsum: W[p, i, kpad56] = sum_s Pi_s * Pj_s
            # Pi is pre-expanded per term (PiX[s][p, (ih, iw, kw7)]) so the
            # kh-peeled multiplies run with step-1 operands (2x bf16 mode).
            # layouts: W/tmp/Y rows are (i, kh, kw) at i*56 + kh*8 + kw with
            # pad column kw=7; the (i,kh) pair merges into one stride-8 dim
            # of 448 (m = 7i + kh), giving 2-dim non-pad views.
            Wv = w_pool.tile([P, EFSP], BF16)
            Wg = wg_pool.tile([P, EFSP], BF16)
            PiX = pix_pool.tile([P, NS * 512], BF16)
            nc.vector.memset(APx(PiX, 7, [[NS * 512, P], [8, NS * 64]]), 0.0)
            for si in range(NS):
                nc.scalar.copy(
                    APx(PiX, si * 512, [[NS * 512, P], [64, 8], [8, 8], [1, 7]]),
                    APx(G9, si * NQ16 + 51, [[G9FS, P], [16, 8], [1, 8], [0, 7]]),
                )

            def term(eng, si, dst):
                for kh in range(7):
                    eng.tensor_tensor(
                        APx(dst, kh * 8, [[EFSP, P], [448, 8], [56, 8], [1, 8]]),
                        APx(PiX, si * 512, [[NS * 512, P], [64, 8], [8, 8], [1, 8]]),
                        APx(G9, si * NQ16 + kh * 16,
                            [[G9FS, P], [16, 8], [1, 8], [1, 8]]),
                        op=mult)

            def flat(t):
                return APx(t, 0, [[EFSP, P], [1, EFSP]])

            # tree-structured accumulation (shorter bf16 error chains)
            term(nc.vector, 0, Wv)
            t1 = tmp_pool.tile([P, EFSP], BF16, tag="tmpd")
            term(nc.vector, 1, t1)
            nc.vector.tensor_tensor(flat(Wv), flat(Wv), flat(t1), op=add)
            u1 = wg_pool.tile([P, EFSP], BF16, tag="wg")
            t2 = tmp_pool.tile([P, EFSP], BF16, tag="tmpd")
            term(nc.vector, 2, u1)
            term(nc.vector, 3, t2)
            nc.vector.tensor_tensor(flat(u1), flat(u1), flat(t2), op=add)
            nc.vector.tensor_tensor(flat(Wv), flat(Wv), flat(u1), op=add)
            u2 = wg_pool.tile([P, EFSP], BF16, tag="wg2")
            t3 = tmp_pool.tile([P, EFSP], BF16, tag="tmpd")
            term(nc.vector, 4, u2)
            term(nc.vector, 5, t3)
            nc.vector.tensor_tensor(flat(u2), flat(u2), flat(t3), op=add)
            u3 = wg_pool.tile([P, EFSP], BF16, tag="wg3")
            t4 = tmp_pool.tile([P, EFSP], BF16, tag="tmpd")
            term(nc.vector, 6, u3)
            term(nc.vector, 7, t4)
            nc.vector.tensor_tensor(flat(u3), flat(u3), flat(t4), op=add)
            nc.vector.tensor_tensor(flat(u2), flat(u2), flat(u3), op=add)
            t5 = tmp_pool.tile([P, EFSP], BF16, tag="tmpd")
            term(nc.vector, 8, t5)
            nc.vector.tensor_tensor(flat(u2), flat(u2), flat(t5), op=add)
            nc.vector.tensor_tensor(flat(Wv), flat(Wv), flat(u2), op=add)

            # ---- per-head: attn -> exp -> y -> sum_k -> normalize -> out
            for hd in range(HD):
                E = e_pool.tile([P, EFS + 8], F32)
                nc.scalar.dma_start(
                    APx(E, 0, [[EFS + 8, P], [1, EFS]]),
                    APx(attn_d, (T * HD + hd) * P * EFS, [[EFS, P], [1, EFS]]),
                )
                nc.vector.memset(APx(E, EFS, [[EFS + 8, P], [1, 8]]), 0.0)
                Eb = eb_pool.tile([P, EFSP], BF16)
                nc.scalar.activation(
                    APx(Eb, 0, [[EFSP, P], [8, 448], [1, 8]]),
                    APx(E, 0, [[EFS + 8, P], [7, 448], [1, 8]]),
                    mybir.ActivationFunctionType.Exp)
                Yp = y_pool.tile([P, EFSP], BF16)
                neng = nc.gpsimd if (T == 0 or hd < 2) else nc.vector
                nc.vector.tensor_tensor(flat(Yp), flat(Eb), flat(Wv), op=mult)
                Ssum = s_pool.tile([P, NI], F32, tag="ssum")
                Rcp = s_pool.tile([P, NI], F32, tag="rcp")
                nc.vector.tensor_reduce(
                    Ssum[:], APx(Yp, 0, [[EFSP, P], [56, NI], [1, 56]]),
                    axis=mybir.AxisListType.X, op=add)
                nc.vector.tensor_scalar_add(Rcp[:], Ssum[:], 1e-15)
                nc.vector.reciprocal(Rcp[:], Rcp[:])
                # normalize, writing f32 compact into the (now free) E tile
                neng.tensor_tensor(
                    APx(E, 0, [[EFS + 8, P], [49, 64], [7, 7], [1, 7]]),
                    APx(Yp, 0, [[EFSP, P], [56, 64], [8, 7], [1, 7]]),
                    APx(Rcp, 0, [[NI, P], [1, NI], [0, 7], [0, 7]]), op=mult)
                nc.sync.dma_start(
                    APx(out_d, (T * HD + hd) * P * EFS, [[EFS, P], [1, EFS]]),
                    APx(E, 0, [[EFS + 8, P], [1, EFS]]),
                )

    nc.compile()
    return nc


def shard_inputs(attn, sims):
    """Full inputs -> per-core in_maps (list of 8 dicts)."""
    attn = np.ascontiguousarray(attn, dtype=np.float32)
    sims = np.ascontiguousarray(sims, dtype=np.float32)
    in_maps = []
    th = np.arange(5)
    for c in range(N_CORES):
        b, j = divmod(c, 4)
        a = attn[b, :, 64 * j:64 * j + 64]            # (hd, 64, 256, 49)
        a = a.reshape(HD, NT, HBT, 8, NBW, 8, K)
        a = a.transpose(1, 0, 2, 4, 3, 5, 6)          # T, hd, hbl, wb, ih, iw, k
        attn_shard = np.ascontiguousarray(a.reshape(NT, HD, P, EFS))
        gpos = np.arange(64 * j - HALO, 64 * j + BAND + HALO)
        gval = np.clip(gpos, 0, H - 1)
        rows = sims[b, gval]                          # (70, 256, 32, 32)
        sh = (gpos[:, None] // 8) + th[None, :] - 2   # (70, 5)
        valid = (sh >= 0) & (sh < SH)
        shc = np.clip(sh, 0, SH - 1)
        slab = np.take_along_axis(rows, shc[:, None, :, None], axis=2)
        slab = np.where(valid[:, None, :, None], slab, np.float32(0.0))
        import ml_dtypes
        in_maps.append({"attn": attn_shard,
                        "slab": np.ascontiguousarray(slab.astype(ml_dtypes.bfloat16))})
    return in_maps


def unshard_output(results):
    out = np.empty((B, HD, H, W, K), dtype=np.float32)
    for c in range(N_CORES):
        b, j = divmod(c, 4)
        o = results[c]["out"].reshape(NT, HD, HBT, NBW, 8, 8, K)
        o = o.transpose(1, 0, 2, 4, 3, 5, 6)          # hd, T, hbl, ih, wb, iw, k
        out[b, :, 64 * j:64 * j + 64] = o.reshape(HD, BAND, W, K)
    return out


_NC_CACHE = {}


def kernel(attn, sims):
    from concourse.bass_utils import run_bass_kernel_spmd
    if "nc" not in _NC_CACHE:
        _NC_CACHE["nc"] = build_graph()
    nc = _NC_CACHE["nc"]
    in_maps = shard_inputs(attn, sims)
    res = run_bass_kernel_spmd(nc, in_maps, core_ids=list(range(N_CORES)))
    return unshard_output(res.results)



# revision 2
# speedup vs baseline: 1.2913x; 1.2913x over previous
"""Trainium2 Bass kernel for nn_AttnReweight (superpixel-reweighted attention).

Math (per batch b, head hd, pixel (h,w), key k in a 7x7 window):
    w[h,w,k] = sum_{s in 3x3 superpixel nbhd} Pi[h,w,s] * Pj[s,h,w,k]
    out = (w * exp(attn)) / sum_k (w * exp(attn))
(The reference's max-shift cancels in the ratio; attn ~ N(0,1) so exp() is
safe without it. eps=1e-15 is negligible vs the denominator ~O(10).)

Sharding: 8 cores = 2 batches x 4 row-bands of 64 rows. Per-core layout
(all host-prepped, all bf16):
  - attn shard in k-major order: [T, hd, p, k*64 + i] where p = (hbl, wb)
    indexes the 128 8x8-pixel blocks of a 32-row tile half, i = (ih, iw)
    the pixel within the block, k = (kh, kw) the key offset. k-major makes
    every on-device elementwise op a packed-bf16 (2x DVE) op, including
    the per-pixel normalize broadcast (Rcp innermost-i, stride-0 over k).
  - G9: per block the 9 superpixel-neighbor planes over the block's 14x14
    key region (8x8 block + 3 halo), rows padded to 16, zero where the
    superpixel falls outside the 32x32 grid. The query-pixel factor Pi is
    G9's center 8x8 window, read with stride-0 broadcast dims - no
    separate Pi tensor on device.
On-device: the 9-term einsum (DVE with a GPSIMD subtree), then per head
exp (ACT) -> Y = E*W (DVE) -> k-reduce (DVE) -> reciprocal -> normalize
(DVE 2x) -> bf16 store. Output unshard + fp32 cast on host.
"""

import sys

sys.path.insert(0, "/opt/trn_rl_repo")

import numpy as np

import concourse.bass as bass
import concourse.tile as tile
from concourse import bacc, mybir
from contextlib import ExitStack

F32 = mybir.dt.float32
BF16 = mybir.dt.bfloat16

# problem geometry (hardcoded per the harness contract)
B, HD, H, W, K = 2, 4, 256, 256, 49
SH = SW = 32
N_CORES = 8
BAND = 64                 # pixel rows per core
NT = 2                    # tile halves (32 rows each) per core
P = 128                   # blocks per tile: 4 block-rows x 32 block-cols
NI = 64                   # pixels per block (8x8)
F = K * NI                # 3136 free elements per (tile, head)
RP = 16                   # G9 region row pitch (14 cols + 2 zero)
SSZ = 14 * RP             # 224 per superpixel plane
G9F = 9 * SSZ             # 2016
CTR = 3 * RP + 3          # 51: center (query) window offset in a plane

mult, add = mybir.AluOpType.mult, mybir.AluOpType.add


def APx(t, off, dims):
    return bass.AP(t.tensor, off, [list(d) for d in dims])


def build_graph():
    nc = bacc.Bacc("TRN2", target_bir_lowering=False, debug=False,
                   num_devices=N_CORES)
    attn_d = nc.dram_tensor("attn", [NT * HD, P, F], BF16,
                            kind="ExternalInput").ap()
    g9_d = nc.dram_tensor("g9", [NT, P, G9F], BF16, kind="ExternalInput").ap()
    out_d = nc.dram_tensor("out", [NT * HD, P, F], BF16,
                           kind="ExternalOutput").ap()

    with tile.TileContext(nc) as tc, ExitStack() as ctx:
        g9_pool = ctx.enter_context(tc.tile_pool(name="g9", bufs=2))
        w_pool = ctx.enter_context(tc.tile_pool(name="wv", bufs=2))
        va_pool = ctx.enter_context(tc.tile_pool(name="va", bufs=1))
        vb_pool = ctx.enter_context(tc.tile_pool(name="vb", bufs=1))
        vc_pool = ctx.enter_context(tc.tile_pool(name="vc", bufs=1))
        gu_pool = ctx.enter_context(tc.tile_pool(name="gu", bufs=2))
        gv_pool = ctx.enter_context(tc.tile_pool(name="gv", bufs=2))
        gw_pool = ctx.enter_context(tc.tile_pool(name="gw", bufs=2))
        e_pool = ctx.enter_context(tc.tile_pool(name="e", bufs=2))
        x_pool = ctx.enter_context(tc.tile_pool(name="x", bufs=2))
        y_pool = ctx.enter_context(tc.tile_pool(name="y", bufs=2))
        d_pool = ctx.enter_context(tc.tile_pool(name="d", bufs=2))
        r_pool = ctx.enter_context(tc.tile_pool(name="r", bufs=2))
        rb_pool = ctx.enter_context(tc.tile_pool(name="rb", bufs=2))
        o_pool = ctx.enter_context(tc.tile_pool(name="o", bufs=2))

        # load both G9 tiles up front (GPSIMD starts tile-1 work early)
        G9s = []
        for T in range(NT):
            G9 = g9_pool.tile([P, G9F], BF16)
            nc.sync.dma_start(G9[:], APx(g9_d, T * P * G9F, [[G9F, P], [1, G9F]]))
            G9s.append(G9)

        def term(eng, G9, si, dst):
            """dst[kh,kw,ih,iw] = Pi[ih,iw] * Pj[ih+kh, iw+kw] for plane si.
            kh is peeled (3 free dims per instruction)."""
            for kh in range(7):
                eng.tensor_tensor(
                    APx(dst, kh * 448, [[F, P], [64, 7], [8, 8], [1, 8]]),
                    APx(G9, si * SSZ + CTR, [[G9F, P], [0, 7], [16, 8], [1, 8]]),
                    APx(G9, si * SSZ + kh * RP,
                        [[G9F, P], [1, 7], [16, 8], [1, 8]]),
                    op=mult)

        def flat(t):
            return APx(t, 0, [[F, P], [1, F]])

        def tt(eng, dst, a, b, op):
            eng.tensor_tensor(flat(dst), flat(a), flat(b), op=op)

        # einsum per tile.  GPSIMD computes a subtree of terms (small for
        # tile 0 so the first Wv lands fast; larger for tile 1 while the
        # vector engine runs tile 0's per-head phase).  DVE does the rest.
        Wvs = []
        gp_terms = {0: (7, 8), 1: (5, 6, 7, 8)}
        for T in range(NT):
            G9 = G9s[T]
            U = gu_pool.tile([P, F], BF16, tag="gu")
            Vg = gv_pool.tile([P, F], BF16, tag="gv")
            ts = gp_terms[T]
            term(nc.gpsimd, G9, ts[0], U)
            term(nc.gpsimd, G9, ts[1], Vg)
            tt(nc.gpsimd, U, U, Vg, add)
            if len(ts) == 4:
                Wg = gw_pool.tile([P, F], BF16, tag="gw")
                term(nc.gpsimd, G9, ts[2], Vg)
                term(nc.gpsimd, G9, ts[3], Wg)
                tt(nc.gpsimd, Vg, Vg, Wg, add)
                tt(nc.gpsimd, U, U, Vg, add)

            dve_ts = [s for s in range(9) if s not in ts]
            A = va_pool.tile([P, F], BF16, tag="va")
            Bv = vb_pool.tile([P, F], BF16, tag="vb")
            Cv = vc_pool.tile([P, F], BF16, tag="vc")
            # tree: A = (t0+t1) + (t2+t3) [+ t4 [+ (t5+t6)]]
            term(nc.vector, G9, dve_ts[0], A)
            term(nc.vector, G9, dve_ts[1], Bv)
            tt(nc.vector, A, A, Bv, add)
            term(nc.vector, G9, dve_ts[2], Bv)
            term(nc.vector, G9, dve_ts[3], Cv)
            tt(nc.vector, Bv, Bv, Cv, add)
            tt(nc.vector, A, A, Bv, add)
            if len(dve_ts) >= 5:
                term(nc.vector, G9, dve_ts[4], Bv)
                if len(dve_ts) == 7:
                    term(nc.vector, G9, dve_ts[5], Cv)
                    tt(nc.vector, Bv, Bv, Cv, add)
                    term(nc.vector, G9, dve_ts[6], Cv)
                    tt(nc.vector, Bv, Bv, Cv, add)
                tt(nc.vector, A, A, Bv, add)
            Wv = w_pool.tile([P, F], BF16)
            tt(nc.vector, Wv, A, U, add)
            Wvs.append(Wv)

        # per-(tile, head) phase, software-pipelined so the normalize of
        # step u-1 issues between Y and D of step u (hides the ACT round
        # trip of the bf16 reciprocal copy).
        pend = None  # (Y, Rb, out_offset)

        def emit_norm(p):
            Yp, Rbp, off = p
            O = o_pool.tile([P, F], BF16)
            nc.vector.tensor_tensor(
                APx(O, 0, [[F, P], [64, 49], [1, 64]]),
                APx(Yp, 0, [[F, P], [64, 49], [1, 64]]),
                APx(Rbp, 0, [[NI, P], [0, 49], [1, 64]]),
                op=mult)
            nc.sync.dma_start(APx(out_d, off, [[F, P], [1, F]]), flat(O))

        for T in range(NT):
            Wv = Wvs[T]
            for hd in range(HD):
                off = (T * HD + hd) * P * F
                Eb = e_pool.tile([P, F], BF16)
                nc.sync.dma_start(flat(Eb), APx(attn_d, off, [[F, P], [1, F]]))
                Ex = x_pool.tile([P, F], BF16)
                nc.scalar.activation(flat(Ex), flat(Eb),
                                     mybir.ActivationFunctionType.Exp)
                Y = y_pool.tile([P, F], BF16)
                tt(nc.vector, Y, Ex, Wv, mult)
                if pend is not None:
                    emit_norm(pend)
                D = d_pool.tile([P, NI], F32, tag="d")
                nc.vector.tensor_reduce(
                    D[:], APx(Y, 0, [[F, P], [1, 64], [64, 49]]),
                    axis=mybir.AxisListType.X, op=add)
                R = r_pool.tile([P, NI], F32, tag="r")
                nc.vector.reciprocal(R[:], D[:])
                Rb = rb_pool.tile([P, NI], BF16, tag="rb")
                nc.scalar.copy(Rb[:], R[:])
                pend = (Y, Rb, off)
        emit_norm(pend)

    nc.compile()
    return nc


def shard_inputs(attn, sims):
    """Full inputs -> per-core in_maps (list of 8 dicts)."""
    import ml_dtypes
    attn = np.ascontiguousarray(attn, dtype=np.float32)
    sims = np.ascontiguousarray(sims, dtype=np.float32)
    in_maps = []
    rh = np.arange(14)
    dhw = np.arange(3) - 1
    for c in range(N_CORES):
        b, j = divmod(c, 4)
        # attn: (hd, 64, 256, 49) -> [T, hd, p=(hbl,wb), k, i=(ih,iw)]
        a = attn[b, :, BAND * j:BAND * j + BAND]
        a = a.reshape(HD, NT, 4, 8, 32, 8, K)        # hd T hbl ih wb iw k
        a = a.transpose(1, 0, 2, 4, 6, 3, 5)         # T hd hbl wb k ih iw
        attn_shard = np.ascontiguousarray(
            a.reshape(NT * HD, P, F).astype(ml_dtypes.bfloat16))

        # G9: [T, p=(hbl,wb), s=(dh,dw), rh, rw(16)]
        sb = sims[b]                                  # (256,256,32,32)
        gbr = (8 * j + 4 * np.arange(NT)[:, None]
               + np.arange(4)[None, :])               # (T, hbl) block rows
        gh = np.clip(gbr[:, :, None] * 8 + rh[None, None, :] - 3,
                     0, H - 1)                        # (T, hbl, 14)
        gw = np.clip(np.arange(32)[:, None] * 8 + rh[None, :] - 3,
                     0, W - 1)                        # (wb, 14)
        sph = gbr[:, :, None] + dhw[None, None, :]    # (T, hbl, 3)
        spw = np.arange(32)[:, None] + dhw[None, :]   # (wb, 3)
        vh = (sph >= 0) & (sph < SH)
        vw = (spw >= 0) & (spw < SW)
        sphc = np.clip(sph, 0, SH - 1)
        spwc = np.clip(spw, 0, SW - 1)
        # gather -> (T, hbl, wb, dh, dw, rh, rw)
        g = sb[gh[:, :, None, None, None, :, None],
               gw[None, None, :, None, None, None, :],
               sphc[:, :, None, :, None, None, None],
               spwc[None, None, :, None, :, None, None]]
        g *= (vh[:, :, None, :, None, None, None]
              & vw[None, None, :, None, :, None, None])
        g9 = np.zeros((NT, 4, 32, 3, 3, 14, RP), dtype=ml_dtypes.bfloat16)
        g9[..., :14] = g
        in_maps.append({"attn": attn_shard,
                        "g9": np.ascontiguousarray(g9.reshape(NT, P, G9F))})
    return in_maps


def unshard_output(results):
    out = np.empty((B, HD, H, W, K), dtype=np.float32)
    for c in range(N_CORES):
        b, j = divmod(c, 4)
        o = results[c]["out"].astype(np.float32)
        o = o.reshape(NT, HD, 4, 32, K, 8, 8)        # T hd hbl wb k ih iw
        o = o.transpose(1, 0, 2, 5, 3, 6, 4)         # hd T hbl ih wb iw k
        out[b, :, BAND * j:BAND * j + BAND] = o.reshape(HD, BAND, W, K)
    return out


_NC_CACHE = {}


def kernel(attn, sims):
    from concourse.bass_utils import run_bass_kernel_spmd
    if "nc" not in _NC_CACHE:
        _NC_CACHE["nc"] = build_graph()
    nc = _NC_CACHE["nc"]
    in_maps = shard_inputs(attn, sims)
    res = run_bass_kernel_spmd(nc, in_maps, core_ids=list(range(N_CORES)))
    return unshard_output(res.results)


# revision 4
# speedup vs baseline: 1.8208x; 1.4100x over previous
"""Trainium2 Bass kernel for nn_AttnReweight (superpixel-reweighted attention).

Math (per batch b, head hd, pixel (h,w), key k in a 7x7 window):
    w[h,w,k] = sum_{s in 3x3 superpixel nbhd} Pi[h,w,s] * Pj[s,h,w,k]
    out = (w * exp(attn)) / sum_k (w * exp(attn))
(The reference's max-shift cancels in the ratio; attn ~ N(0,1) so exp() is
safe without it. eps=1e-15 is negligible vs the denominator ~O(10).)

Sharding: 8 cores = 2 batches x 4 row-bands of 64 rows. Per-core, all
host-prepped gathers, all bf16, k-major free layout [k*64 + i] (k = key
offset in the 7x7 window, i = pixel in the 8x8 block; p = 128 blocks of a
32-row tile half).  k-major makes every device op a packed unit-stride
DVE op, including the per-pixel normalize broadcast (stride-0 over k,
innermost i).

  - attn shard: [T, hd, p, k*64+i]
  - PjX: the superpixel factor at the key pixel, pre-expanded per term:
    PjX[T, p, s, k*64+i] = sims[b, hj, wj, sph(s), spw(s)] (zero outside
    the 32x32 superpixel grid).  Uploading the expansion (pure gather)
    makes each of the 9 einsum multiplies one flat 3136-element
    instruction - the ISA's 3-free-dim AP limit makes the in-place
    windowed read 7x more instructions, and GPSIMD offload poisons DVE
    throughput via SBUF contention, so flat DVE-only is fastest.
  - PiC: the query-pixel factor, compact: PiC[T, p, s*64+i].

On-device per tile: 9 flat multiplies + 8 flat tree-adds -> Wv; per head
exp (ACT) -> Y = E*W -> k-reduce -> reciprocal (-> bf16 on ACT) ->
normalize -> bf16 store.  Output unshard + fp32 cast on host.
"""

import sys

sys.path.insert(0, "/opt/trn_rl_repo")

import numpy as np

import concourse.bass as bass
import concourse.tile as tile
from concourse import bacc, mybir
from contextlib import ExitStack

F32 = mybir.dt.float32
BF16 = mybir.dt.bfloat16

# problem geometry (hardcoded per the harness contract)
B, HD, H, W, K = 2, 4, 256, 256, 49
SH = SW = 32
N_CORES = 8
BAND = 64                 # pixel rows per core
NT = 2                    # tile halves (32 rows each) per core
P = 128                   # blocks per tile: 4 block-rows x 32 block-cols
NI = 64                   # pixels per block (8x8)
F = K * NI                # 3136 free elements per (tile, head)
NS = 9
GSZ = 3 * F               # PjX upload chunk: 3 superpixel terms

mult, add = mybir.AluOpType.mult, mybir.AluOpType.add


def APx(t, off, dims):
    return bass.AP(t.tensor, off, [list(d) for d in dims])


def build_graph():
    nc = bacc.Bacc("TRN2", target_bir_lowering=False, debug=False,
                   num_devices=N_CORES)
    attn_d = nc.dram_tensor("attn", [NT * HD, P, F], BF16,
                            kind="ExternalInput").ap()
    pjx_d = nc.dram_tensor("pjx", [NT, 3, P, GSZ], BF16,
                           kind="ExternalInput").ap()
    pic_d = nc.dram_tensor("pic", [NT, P, NS * NI], BF16,
                           kind="ExternalInput").ap()
    out_d = nc.dram_tensor("out", [NT * HD, P, F], BF16,
                           kind="ExternalOutput").ap()

    with tile.TileContext(nc) as tc, ExitStack() as ctx:
        pj_pool = ctx.enter_context(tc.tile_pool(name="pjx", bufs=3))
        pi_pool = ctx.enter_context(tc.tile_pool(name="pic", bufs=2))
        w_pool = ctx.enter_context(tc.tile_pool(name="wv", bufs=2))
        va_pool = ctx.enter_context(tc.tile_pool(name="va", bufs=1))
        vb_pool = ctx.enter_context(tc.tile_pool(name="vb", bufs=1))
        vc_pool = ctx.enter_context(tc.tile_pool(name="vc", bufs=1))
        e_pool = ctx.enter_context(tc.tile_pool(name="e", bufs=2))
        x_pool = ctx.enter_context(tc.tile_pool(name="x", bufs=2))
        y_pool = ctx.enter_context(tc.tile_pool(name="y", bufs=2))
        d_pool = ctx.enter_context(tc.tile_pool(name="d", bufs=2))
        r_pool = ctx.enter_context(tc.tile_pool(name="r", bufs=2))
        rb_pool = ctx.enter_context(tc.tile_pool(name="rb", bufs=2))
        o_pool = ctx.enter_context(tc.tile_pool(name="o", bufs=2))

        def flat(t):
            return APx(t, 0, [[F, P], [1, F]])

        def tt(dst, a, b, op):
            nc.vector.tensor_tensor(flat(dst), flat(a), flat(b), op=op)

        def fetch_tile_inputs(T):
            chunks = []
            for g in range(3):
                PJ = pj_pool.tile([P, GSZ], BF16, tag="pjx")
                nc.sync.dma_start(
                    PJ[:], APx(pjx_d, (T * 3 + g) * P * GSZ,
                               [[GSZ, P], [1, GSZ]]))
                chunks.append(PJ)
            PI = pi_pool.tile([P, NS * NI], BF16, tag="pic")
            nc.sync.dma_start(
                PI[:], APx(pic_d, T * P * NS * NI,
                           [[NS * NI, P], [1, NS * NI]]))
            return chunks, PI

        def einsum(chunks, PI):
            """Wv = sum_s PiC[s] * PjX[s], 9 flat mults + 8 flat adds."""
            def term(si, dst):
                nc.vector.tensor_tensor(
                    flat(dst),
                    APx(PI, si * NI, [[NS * NI, P], [0, K], [1, NI]]),
                    APx(chunks[si // 3], (si % 3) * F, [[GSZ, P], [1, F]]),
                    op=mult)
            A = va_pool.tile([P, F], BF16, tag="va")
            Bv = vb_pool.tile([P, F], BF16, tag="vb")
            Cv = vc_pool.tile([P, F], BF16, tag="vc")
            term(0, A)
            term(1, Bv)
            tt(A, A, Bv, add)
            term(2, Bv)
            term(3, Cv)
            tt(Bv, Bv, Cv, add)
            tt(A, A, Bv, add)
            term(4, Bv)
            term(5, Cv)
            tt(Bv, Bv, Cv, add)
            term(6, Cv)
            tt(Bv, Bv, Cv, add)
            tt(A, A, Bv, add)
            term(7, Bv)
            term(8, Cv)
            tt(Bv, Bv, Cv, add)
            Wv = w_pool.tile([P, F], BF16)
            tt(Wv, A, Bv, add)
            return Wv

        # tile 0 inputs + einsum
        chunks, PI = fetch_tile_inputs(0)
        Wvs = [einsum(chunks, PI), None]

        # per-(tile, head) phase, software-pipelined: the normalize of step
        # u-1 issues between Y and D of step u (hides the ACT round trip of
        # the bf16 reciprocal copy); tile 1's inputs stream during tile 0's
        # head phase and its einsum runs right after.
        pend = None  # (Y, Rb, out_offset)

        def emit_norm(p):
            Yp, Rbp, off = p
            O = o_pool.tile([P, F], BF16)
            nc.vector.tensor_tensor(
                APx(O, 0, [[F, P], [64, K], [1, NI]]),
                APx(Yp, 0, [[F, P], [64, K], [1, NI]]),
                APx(Rbp, 0, [[NI, P], [0, K], [1, NI]]),
                op=mult)
            nc.sync.dma_start(APx(out_d, off, [[F, P], [1, F]]), flat(O))

        for T in range(NT):
            Wv = Wvs[T]
            for hd in range(HD):
                off = (T * HD + hd) * P * F
                Eb = e_pool.tile([P, F], BF16)
                nc.sync.dma_start(flat(Eb), APx(attn_d, off, [[F, P], [1, F]]))
                if T == 0 and hd < 3:
                    # stream tile 1's inputs behind the head-phase loads
                    if hd == 0:
                        c1, PI1 = fetch_tile_inputs(1)
                Ex = x_pool.tile([P, F], BF16)
                nc.scalar.activation(flat(Ex), flat(Eb),
                                     mybir.ActivationFunctionType.Exp)
                Y = y_pool.tile([P, F], BF16)
                tt(Y, Ex, Wv, mult)
                if pend is not None:
                    emit_norm(pend)
                D = d_pool.tile([P, NI], F32, tag="d")
                nc.vector.tensor_reduce(
                    D[:], APx(Y, 0, [[F, P], [1, NI], [NI, K]]),
                    axis=mybir.AxisListType.X, op=add)
                R = r_pool.tile([P, NI], F32, tag="r")
                nc.vector.reciprocal(R[:], D[:])
                Rb = rb_pool.tile([P, NI], BF16, tag="rb")
                nc.scalar.copy(Rb[:], R[:])
                pend = (Y, Rb, off)
            if T == 0:
                Wvs[1] = einsum(c1, PI1)
        emit_norm(pend)

    nc.compile()
    return nc


def shard_inputs(attn, sims):
    """Full inputs -> per-core in_maps (list of 8 dicts)."""
    import ml_dtypes
    attn = np.ascontiguousarray(attn, dtype=np.float32)
    sims = np.ascontiguousarray(sims, dtype=np.float32)
    in_maps = []
    rh = np.arange(14)
    dhw = np.arange(3) - 1
    for c in range(N_CORES):
        b, j = divmod(c, 4)
        # attn: (hd, 64, 256, 49) -> [T, hd, p=(hbl,wb), k, i=(ih,iw)]
        a = attn[b, :, BAND * j:BAND * j + BAND]
        a = a.reshape(HD, NT, 4, 8, 32, 8, K)        # hd T hbl ih wb iw k
        a = a.transpose(1, 0, 2, 4, 6, 3, 5)         # T hd hbl wb k ih iw
        attn_shard = np.ascontiguousarray(
            a.reshape(NT * HD, P, F).astype(ml_dtypes.bfloat16))

        # superpixel-factor gather over the 14x14 region per block
        sb = sims[b]                                  # (256,256,32,32)
        gbr = (8 * j + 4 * np.arange(NT)[:, None]
               + np.arange(4)[None, :])               # (T, hbl) block rows
        gh = np.clip(gbr[:, :, None] * 8 + rh[None, None, :] - 3,
                     0, H - 1)                        # (T, hbl, 14)
        gw = np.clip(np.arange(32)[:, None] * 8 + rh[None, :] - 3,
                     0, W - 1)                        # (wb, 14)
        sph = gbr[:, :, None] + dhw[None, None, :]    # (T, hbl, 3)
        spw = np.arange(32)[:, None] + dhw[None, :]   # (wb, 3)
        vh = (sph >= 0) & (sph < SH)
        vw = (spw >= 0) & (spw < SW)
        sphc = np.clip(sph, 0, SH - 1)
        spwc = np.clip(spw, 0, SW - 1)
        # g: (T, hbl, wb, dh, dw, rh14, rw14)
        g = sb[gh[:, :, None, None, None, :, None],
               gw[None, None, :, None, None, None, :],
               sphc[:, :, None, :, None, None, None],
               spwc[None, None, :, None, :, None, None]]
        g *= (vh[:, :, None, :, None, None, None]
              & vw[None, None, :, None, :, None, None])
        # PiC[T, p, s, i]: center 8x8 of each region
        pic = np.ascontiguousarray(
            g[..., 3:11, 3:11].reshape(NT, P, NS * NI)
        ).astype(ml_dtypes.bfloat16)
        # PjX[T, p, s, k, i]: 7x7 sliding windows, k-major
        wnd = np.lib.stride_tricks.sliding_window_view(g, (7, 7), axis=(5, 6))
        # wnd: (T, hbl, wb, dh, dw, ih8, iw8, kh7, kw7)
        pjx = wnd.transpose(0, 1, 2, 3, 4, 7, 8, 5, 6)  # ... kh kw ih iw
        pjx = pjx.reshape(NT, P, 3, 3, F).transpose(0, 2, 1, 3, 4)
        pjx = np.ascontiguousarray(
            pjx.reshape(NT, 3, P, GSZ)).astype(ml_dtypes.bfloat16)
        in_maps.append({"attn": attn_shard, "pjx": pjx, "pic": pic})
    return in_maps


def unshard_output(results):
    out = np.empty((B, HD, H, W, K), dtype=np.float32)
    for c in range(N_CORES):
        b, j = divmod(c, 4)
        o = results[c]["out"].astype(np.float32)
        o = o.reshape(NT, HD, 4, 32, K, 8, 8)        # T hd hbl wb k ih iw
        o = o.transpose(1, 0, 2, 5, 3, 6, 4)         # hd T hbl ih wb iw k
        out[b, :, BAND * j:BAND * j + BAND] = o.reshape(HD, BAND, W, K)
    return out


_NC_CACHE = {}


def kernel(attn, sims):
    from concourse.bass_utils import run_bass_kernel_spmd
    if "nc" not in _NC_CACHE:
        _NC_CACHE["nc"] = build_graph()
    nc = _NC_CACHE["nc"]
    in_maps = shard_inputs(attn, sims)
    res = run_bass_kernel_spmd(nc, in_maps, core_ids=list(range(N_CORES)))
    return unshard_output(res.results)


# revision 8
# speedup vs baseline: 2.1520x; 1.1819x over previous
"""Trainium2 Bass kernel for nn_AttnReweight (superpixel-reweighted attention).

Math (per batch b, head hd, pixel (h,w), key k in a 7x7 window):
    w[h,w,k] = sum_{s in 3x3 superpixel nbhd} Pi[h,w,s] * Pj[s,h,w,k]
    out = (w * exp(attn)) / sum_k (w * exp(attn))
(The reference's max-shift cancels in the ratio; attn ~ N(0,1) so exp() is
safe without it. eps=1e-15 is negligible vs the denominator ~O(10).)

Sharding: 8 cores = 2 batches x 4 row-bands of 64 rows. Per-core, all
host-prepped gathers, all bf16, k-major free layout [k*64 + i] (k = key
offset in the 7x7 window, i = pixel in the 8x8 block; p = 128 blocks of a
32-row tile half).  k-major makes every device op a packed unit-stride
DVE op, including the per-pixel normalize broadcast (stride-0 over k,
innermost i) and the k-reduction fold tree (stride-64*n outer, packed-64
inner).

  - attn shard: [T, hd, p, k*64+i]
  - PjX: the superpixel factor at the key pixel, pre-expanded per term:
    PjX[T, p, s, k*64+i] = sims[b, hj, wj, sph(s), spw(s)] (zero outside
    the 32x32 superpixel grid).  Uploading the expansion (pure gather)
    makes each einsum multiply a flat unit-stride instruction - the
    ISA's 3-free-dim AP limit makes the in-place windowed read 7x more
    instructions, and GPSIMD offload poisons DVE throughput via SBUF
    contention, so flat DVE-only is fastest.  Chunked 4+4+1 terms so the
    first multiply starts ~3us in and mults batch 4 terms/instruction.
  - PiC: the query-pixel factor, compact: PiC[T, p, s*64+i].

On-device per tile: einsum -> Wv (batched flat mults + adds); per head
exp (ACT) -> Y = E*W -> k-fold-tree -> reciprocal (-> bf16 on ACT) ->
normalize -> bf16 store.  Output unshard + fp32 cast on host.
"""

import sys

sys.path.insert(0, "/opt/trn_rl_repo")

import numpy as np

import concourse.bass as bass
import concourse.tile as tile
from concourse import bacc, mybir
from contextlib import ExitStack

F32 = mybir.dt.float32
BF16 = mybir.dt.bfloat16

# problem geometry (hardcoded per the harness contract)
B, HD, H, W, K = 2, 4, 256, 256, 49
SH = SW = 32
N_CORES = 8
BAND = 64                 # pixel rows per core
NT = 2                    # tile halves (32 rows each) per core
P = 128                   # blocks per tile: 4 block-rows x 32 block-cols
NI = 64                   # pixels per block (8x8)
F = K * NI                # 3136 free elements per (tile, head)
NS = 9

mult, add = mybir.AluOpType.mult, mybir.AluOpType.add


def APx(t, off, dims):
    return bass.AP(t.tensor, off, [list(d) for d in dims])


def build_graph():
    nc = bacc.Bacc("TRN2", target_bir_lowering=False, debug=False,
                   num_devices=N_CORES)
    attn_d = nc.dram_tensor("attn", [NT * HD, P, F], BF16,
                            kind="ExternalInput").ap()
    # PjX split 4+4+1 terms per tile
    pjq_d = nc.dram_tensor("pjq", [NT, 2, P, 4 * F], BF16,
                           kind="ExternalInput").ap()
    pjs_d = nc.dram_tensor("pjs", [NT, P, F], BF16, kind="ExternalInput").ap()
    pic_d = nc.dram_tensor("pic", [NT, P, NS * NI], BF16,
                           kind="ExternalInput").ap()
    out_d = nc.dram_tensor("out", [NT * HD, P, F], BF16,
                           kind="ExternalOutput").ap()

    with tile.TileContext(nc) as tc, ExitStack() as ctx:
        pjq_pool = ctx.enter_context(tc.tile_pool(name="pjq", bufs=1))
        pjs_pool = ctx.enter_context(tc.tile_pool(name="pjs", bufs=1))
        pi_pool = ctx.enter_context(tc.tile_pool(name="pic", bufs=2))
        w_pool = ctx.enter_context(tc.tile_pool(name="wv", bufs=2))
        q1_pool = ctx.enter_context(tc.tile_pool(name="q1", bufs=1))
        q2_pool = ctx.enter_context(tc.tile_pool(name="q2", bufs=1))
        va_pool = ctx.enter_context(tc.tile_pool(name="va", bufs=1))
        e_pool = ctx.enter_context(tc.tile_pool(name="e", bufs=2))
        x_pool = ctx.enter_context(tc.tile_pool(name="x", bufs=2))
        y_pool = ctx.enter_context(tc.tile_pool(name="y", bufs=2))
        f_pool = ctx.enter_context(tc.tile_pool(name="fold", bufs=2))
        d_pool = ctx.enter_context(tc.tile_pool(name="d", bufs=2))
        r_pool = ctx.enter_context(tc.tile_pool(name="r", bufs=2))
        rb_pool = ctx.enter_context(tc.tile_pool(name="rb", bufs=2))
        o_pool = ctx.enter_context(tc.tile_pool(name="o", bufs=2))

        def flat(t, off=0, n=F):
            return APx(t, off, [[t.tensor.shape[1], P], [1, n]])

        def fetch_tile_inputs(T):
            # PiC first - it gates the first multiply
            PI = pi_pool.tile([P, NS * NI], BF16, tag="pic")
            nc.sync.dma_start(
                PI[:], APx(pic_d, T * P * NS * NI,
                           [[NS * NI, P], [1, NS * NI]]))
            quads = []
            for q in range(2):
                PJ = pjq_pool.tile([P, 4 * F], BF16, tag="pjq")
                nc.sync.dma_start(
                    PJ[:], APx(pjq_d, (T * 2 + q) * P * 4 * F,
                               [[4 * F, P], [1, 4 * F]]))
                quads.append(PJ)
            PS = pjs_pool.tile([P, F], BF16, tag="pjs")
            nc.sync.dma_start(
                PS[:], APx(pjs_d, T * P * F, [[F, P], [1, F]]))
            return quads, PS, PI

        def einsum(quads, PS, PI):
            """Wv = sum_s PiC[s]*PjX[s]; mults batched 4 terms/instr."""
            Q1 = q1_pool.tile([P, 4 * F], BF16, tag="q1")
            Q2 = q2_pool.tile([P, 4 * F], BF16, tag="q2")
            for q, dst in ((0, Q1), (1, Q2)):
                nc.vector.tensor_tensor(
                    APx(dst, 0, [[4 * F, P], [F, 4], [NI, K], [1, NI]]),
                    APx(PI, q * 4 * NI, [[NS * NI, P], [NI, 4], [0, K], [1, NI]]),
                    APx(quads[q], 0, [[4 * F, P], [F, 4], [NI, K], [1, NI]]),
                    op=mult)
            A = va_pool.tile([P, F], BF16, tag="va")
            # t8 = PiC[8] * PjX[8]
            nc.vector.tensor_tensor(
                flat(A),
                APx(PI, 8 * NI, [[NS * NI, P], [0, K], [1, NI]]),
                flat(PS), op=mult)
            # Q1 += Q2 (4 pairwise partial sums)
            nc.vector.tensor_tensor(flat(Q1, 0, 4 * F), flat(Q1, 0, 4 * F),
                                    flat(Q2, 0, 4 * F), op=add)
            # Q2[0:2F] = Q1[0:2F] + Q1[2F:4F]
            nc.vector.tensor_tensor(flat(Q2, 0, 2 * F), flat(Q1, 0, 2 * F),
                                    flat(Q1, 2 * F, 2 * F), op=add)
            # A += Q2[0:F] + Q2[F:2F]
            Wv = w_pool.tile([P, F], BF16)
            nc.vector.tensor_tensor(flat(Wv), flat(Q2, 0, F),
                                    flat(Q2, F, F), op=add)
            nc.vector.tensor_tensor(flat(Wv), flat(Wv), flat(A), op=add)
            return Wv

        # tile 0 inputs + einsum
        quads, PS, PI = fetch_tile_inputs(0)
        Wvs = [einsum(quads, PS, PI), None]

        # per-(tile, head) phase, software-pipelined: the normalize of step
        # u-1 issues between Y and the fold tree of step u (hides the ACT
        # round trip of the bf16 reciprocal copy); tile 1's inputs stream
        # behind the head-phase loads and its einsum runs right after.
        pend = None  # (Y, Rb, out_offset)

        def emit_norm(p):
            Yp, Rbp, off = p
            O = o_pool.tile([P, F], BF16)
            nc.vector.tensor_tensor(
                APx(O, 0, [[F, P], [64, K], [1, NI]]),
                APx(Yp, 0, [[F, P], [64, K], [1, NI]]),
                APx(Rbp, 0, [[NI, P], [0, K], [1, NI]]),
                op=mult)
            nc.sync.dma_start(APx(out_d, off, [[F, P], [1, F]]), flat(O))

        def fold_reduce(Y):
            """D[P,64] f32 = sum_k Y[k*64+i] via a packed-64 TT fold tree:
            48 -> 24 -> 12 -> 6 -> 3 pair-fold columns, then the stray
            col 48 and the last three columns; final add writes fp32."""
            S = f_pool.tile([P, 24 * NI], BF16, tag="fold")
            D = d_pool.tile([P, NI], F32, tag="d")

            def seg(t, c0, n):
                return APx(t, c0 * NI, [[t.tensor.shape[1], P],
                                        [NI, n], [1, NI]])
            nc.vector.tensor_tensor(seg(S, 0, 24), seg(Y, 0, 24),
                                    seg(Y, 24, 24), op=add)
            nc.vector.tensor_tensor(seg(S, 0, 12), seg(S, 0, 12),
                                    seg(S, 12, 12), op=add)
            nc.vector.tensor_tensor(seg(S, 0, 6), seg(S, 0, 6),
                                    seg(S, 6, 6), op=add)
            nc.vector.tensor_tensor(seg(S, 0, 3), seg(S, 0, 3),
                                    seg(S, 3, 3), op=add)
            # live: S[0], S[1], S[2] and Y[48]
            nc.vector.tensor_tensor(seg(S, 0, 1), seg(S, 0, 1),
                                    seg(Y, 48, 1), op=add)
            nc.vector.tensor_tensor(seg(S, 1, 1), seg(S, 1, 1),
                                    seg(S, 2, 1), op=add)
            nc.vector.tensor_tensor(
                APx(D, 0, [[NI, P], [1, NI]]),
                APx(S, 0, [[24 * NI, P], [1, NI]]),
                APx(S, NI, [[24 * NI, P], [1, NI]]), op=add)
            return D

        for T in range(NT):
            Wv = Wvs[T]
            for hd in range(HD):
                off = (T * HD + hd) * P * F
                Eb = e_pool.tile([P, F], BF16)
                nc.sync.dma_start(flat(Eb), APx(attn_d, off, [[F, P], [1, F]]))
                if T == 0 and hd == 0:
                    # stream tile 1's inputs behind the head-phase loads
                    in1 = fetch_tile_inputs(1)
                Ex = x_pool.tile([P, F], BF16)
                nc.scalar.activation(flat(Ex), flat(Eb),
                                     mybir.ActivationFunctionType.Exp)
                Y = y_pool.tile([P, F], BF16)
                nc.vector.tensor_tensor(flat(Y), flat(Ex), flat(Wv), op=mult)
                if pend is not None:
                    emit_norm(pend)
                D = fold_reduce(Y)
                R = r_pool.tile([P, NI], F32, tag="r")
                nc.vector.reciprocal(R[:], D[:])
                Rb = rb_pool.tile([P, NI], BF16, tag="rb")
                nc.scalar.copy(Rb[:], R[:])
                pend = (Y, Rb, off)
            if T == 0:
                Wvs[1] = einsum(*in1)
        emit_norm(pend)

    nc.compile()
    return nc


def shard_inputs(attn, sims):
    """Full inputs -> per-core in_maps (list of 8 dicts)."""
    import ml_dtypes
    attn = np.ascontiguousarray(attn, dtype=np.float32)
    sims = np.ascontiguousarray(sims, dtype=np.float32)
    in_maps = []
    rh = np.arange(14)
    dhw = np.arange(3) - 1
    for c in range(N_CORES):
        b, j = divmod(c, 4)
        # attn: (hd, 64, 256, 49) -> [T, hd, p=(hbl,wb), k, i=(ih,iw)]
        a = attn[b, :, BAND * j:BAND * j + BAND]
        a = a.reshape(HD, NT, 4, 8, 32, 8, K)        # hd T hbl ih wb iw k
        a = a.transpose(1, 0, 2, 4, 6, 3, 5)         # T hd hbl wb k ih iw
        attn_shard = np.ascontiguousarray(
            a.reshape(NT * HD, P, F).astype(ml_dtypes.bfloat16))

        # superpixel-factor gather over the 14x14 region per block
        sb = sims[b]                                  # (256,256,32,32)
        gbr = (8 * j + 4 * np.arange(NT)[:, None]
               + np.arange(4)[None, :])               # (T, hbl) block rows
        gh = np.clip(gbr[:, :, None] * 8 + rh[None, None, :] - 3,
                     0, H - 1)                        # (T, hbl, 14)
        gw = np.clip(np.arange(32)[:, None] * 8 + rh[None, :] - 3,
                     0, W - 1)                        # (wb, 14)
        sph = gbr[:, :, None] + dhw[None, None, :]    # (T, hbl, 3)
        spw = np.arange(32)[:, None] + dhw[None, :]   # (wb, 3)
        vh = (sph >= 0) & (sph < SH)
        vw = (spw >= 0) & (spw < SW)
        sphc = np.clip(sph, 0, SH - 1)
        spwc = np.clip(spw, 0, SW - 1)
        # g: (T, hbl, wb, dh, dw, rh14, rw14)
        g = sb[gh[:, :, None, None, None, :, None],
               gw[None, None, :, None, None, None, :],
               sphc[:, :, None, :, None, None, None],
               spwc[None, None, :, None, :, None, None]]
        g *= (vh[:, :, None, :, None, None, None]
              & vw[None, None, :, None, :, None, None])
        # PiC[T, p, s, i]: center 8x8 of each region
        pic = np.ascontiguousarray(
            g[..., 3:11, 3:11].reshape(NT, P, NS * NI)
        ).astype(ml_dtypes.bfloat16)
        # PjX[T, p, s, k, i]: 7x7 sliding windows, k-major
        wnd = np.lib.stride_tricks.sliding_window_view(g, (7, 7), axis=(5, 6))
        # wnd: (T, hbl, wb, dh, dw, ih8, iw8, kh7, kw7)
        pjx = wnd.transpose(0, 1, 2, 3, 4, 7, 8, 5, 6)  # ... kh kw ih iw
        pjx = pjx.reshape(NT, P, NS, F)
        pjq = np.ascontiguousarray(
            pjx[:, :, :8].reshape(NT, P, 2, 4 * F).transpose(0, 2, 1, 3)
        ).astype(ml_dtypes.bfloat16)
        pjs = np.ascontiguousarray(pjx[:, :, 8]).astype(ml_dtypes.bfloat16)
        in_maps.append({"attn": attn_shard, "pjq": pjq, "pjs": pjs,
                        "pic": pic})
    return in_maps


def unshard_output(results):
    out = np.empty((B, HD, H, W, K), dtype=np.float32)
    for c in range(N_CORES):
        b, j = divmod(c, 4)
        o = results[c]["out"].astype(np.float32)
        o = o.reshape(NT, HD, 4, 32, K, 8, 8)        # T hd hbl wb k ih iw
        o = o.transpose(1, 0, 2, 5, 3, 6, 4)         # hd T hbl ih wb iw k
        out[b, :, BAND * j:BAND * j + BAND] = o.reshape(HD, BAND, W, K)
    return out


_NC_CACHE = {}


def kernel(attn, sims):
    from concourse.bass_utils import run_bass_kernel_spmd
    if "nc" not in _NC_CACHE:
        _NC_CACHE["nc"] = build_graph()
    nc = _NC_CACHE["nc"]
    in_maps = shard_inputs(attn, sims)
    res = run_bass_kernel_spmd(nc, in_maps, core_ids=list(range(N_CORES)))
    return unshard_output(res.results)


# revision 10
# speedup vs baseline: 2.3140x; 1.0753x over previous
"""Trainium2 Bass kernel for nn_AttnReweight (superpixel-reweighted attention).

Math (per batch b, head hd, pixel (h,w), key k in a 7x7 window):
    w[h,w,k] = sum_{s in 3x3 superpixel nbhd} Pi[h,w,s] * Pj[s,h,w,k]
    out = (w * exp(attn)) / sum_k (w * exp(attn))
(The reference's max-shift cancels in the ratio; attn ~ N(0,1) so exp() is
safe without it. eps=1e-15 is negligible vs the denominator ~O(10).)

Sharding: 8 cores = 2 batches x 4 row-bands of 64 rows. Per-core, all
host-prepped gathers, all bf16, k-major free layout [k*64 + i] (k = key
offset in the 7x7 window, i = pixel in the 8x8 block; p = 128 blocks of a
32-row tile half).  k-major makes every device op a packed unit-stride
DVE op, including the per-pixel normalize broadcast (stride-0 over k,
innermost i) and the k-reduction fold tree (packed-64 segments).

  - attn shard: [T, hd, p, k*64+i]
  - PjX: the superpixel factor at the key pixel, pre-expanded per term
    (pure gather): PjX[T, p, s, k*64+i] = sims[b, hj, wj, sph(s), spw(s)],
    zero outside the 32x32 superpixel grid.  Tile 0's nine terms stream
    as single-term chunks (first multiply starts ~3us in); tile 1's
    first eight terms come as two 4-term quads (mults batch 4 terms per
    instruction, multiplied in place over the quad buffer) loaded during
    tile 0's head phase.
  - PiC: the query-pixel factor, compact: PiC[T, p, s*64+i].

Everything computes on DVE except exp / bf16 reciprocal casts (ACT):
GPSIMD work poisons DVE throughput via SBUF contention, and the ISA's
3-free-dim AP limit plus ~225ns/instr overhead favor flat batched ops.
Heads are processed in PAIRS (one instruction covers both heads' grids)
to halve instruction count.  Output unshard + fp32 cast on host.
"""

import sys

sys.path.insert(0, "/opt/trn_rl_repo")

import numpy as np

import concourse.bass as bass
import concourse.tile as tile
from concourse import bacc, mybir
from contextlib import ExitStack

F32 = mybir.dt.float32
BF16 = mybir.dt.bfloat16

# problem geometry (hardcoded per the harness contract)
B, HD, H, W, K = 2, 4, 256, 256, 49
SH = SW = 32
N_CORES = 8
BAND = 64                 # pixel rows per core
NT = 2                    # tile halves (32 rows each) per core
P = 128                   # blocks per tile: 4 block-rows x 32 block-cols
NI = 64                   # pixels per block (8x8)
F = K * NI                # 3136 free elements per (tile, head)
F2 = 2 * F
NS = 9

mult, add = mybir.AluOpType.mult, mybir.AluOpType.add


def APx(t, off, dims):
    return bass.AP(t.tensor, off, [list(d) for d in dims])


def build_graph():
    nc = bacc.Bacc("TRN2", target_bir_lowering=False, debug=False,
                   num_devices=N_CORES)
    attn_d = nc.dram_tensor("attn", [NT * HD, P, F], BF16,
                            kind="ExternalInput").ap()
    pjt_d = nc.dram_tensor("pjt", [NS, P, F], BF16,
                           kind="ExternalInput").ap()   # tile 0, per term
    pjq_d = nc.dram_tensor("pjq", [2, P, 4 * F], BF16,
                           kind="ExternalInput").ap()   # tile 1, quads
    pjs_d = nc.dram_tensor("pjs", [P, F], BF16,
                           kind="ExternalInput").ap()   # tile 1, term 8
    pic_d = nc.dram_tensor("pic", [NT, P, NS * NI], BF16,
                           kind="ExternalInput").ap()
    out_d = nc.dram_tensor("out", [NT * HD, P, F], BF16,
                           kind="ExternalOutput").ap()

    with tile.TileContext(nc) as tc, ExitStack() as ctx:
        pjt_pool = ctx.enter_context(tc.tile_pool(name="pjt", bufs=3))
        pjq_pool = ctx.enter_context(tc.tile_pool(name="pjq", bufs=2))
        pjs_pool = ctx.enter_context(tc.tile_pool(name="pjs", bufs=1))
        pi_pool = ctx.enter_context(tc.tile_pool(name="pic", bufs=2))
        w_pool = ctx.enter_context(tc.tile_pool(name="wv", bufs=2))
        a_pool = ctx.enter_context(tc.tile_pool(name="ea", bufs=1))
        b_pool = ctx.enter_context(tc.tile_pool(name="eb", bufs=1))
        c_pool = ctx.enter_context(tc.tile_pool(name="ec", bufs=1))
        e_pool = ctx.enter_context(tc.tile_pool(name="e2", bufs=2))
        x_pool = ctx.enter_context(tc.tile_pool(name="x2", bufs=1))
        y_pool = ctx.enter_context(tc.tile_pool(name="y2", bufs=2))
        f_pool = ctx.enter_context(tc.tile_pool(name="fold", bufs=2))
        d_pool = ctx.enter_context(tc.tile_pool(name="d2", bufs=2))
        r_pool = ctx.enter_context(tc.tile_pool(name="r2", bufs=2))
        rb_pool = ctx.enter_context(tc.tile_pool(name="rb2", bufs=2))
        o_pool = ctx.enter_context(tc.tile_pool(name="o2", bufs=1))

        def flat(t, off=0, n=F):
            return APx(t, off, [[t.tensor.shape[1], P], [1, n]])

        def tta(dst, a, b):
            nc.vector.tensor_tensor(flat(dst), flat(a), flat(b), op=add)

        def pi_bcast(PI, si, ns=1):
            if ns == 1:
                return APx(PI, si * NI, [[NS * NI, P], [0, K], [1, NI]])
            return APx(PI, si * NI, [[NS * NI, P], [NI, ns], [0, K], [1, NI]])

        # ---- tile 0: PiC + nine streamed single-term chunks
        PI0 = pi_pool.tile([P, NS * NI], BF16, tag="pic")
        nc.sync.dma_start(
            PI0[:], APx(pic_d, 0, [[NS * NI, P], [1, NS * NI]]))
        chunks = []
        for si in range(NS):
            CH = pjt_pool.tile([P, F], BF16, tag="pjt")
            nc.sync.dma_start(
                CH[:], APx(pjt_d, si * P * F, [[F, P], [1, F]]))
            chunks.append(CH)

        def term0(si, dst):
            nc.vector.tensor_tensor(flat(dst), pi_bcast(PI0, si),
                                    flat(chunks[si]), op=mult)

        A = a_pool.tile([P, F], BF16, tag="a")
        Bv = b_pool.tile([P, F], BF16, tag="b")
        Cv = c_pool.tile([P, F], BF16, tag="c")
        term0(0, A)
        term0(1, Bv)
        tta(A, A, Bv)                 # A = t01
        term0(2, Bv)
        term0(3, Cv)
        tta(Bv, Bv, Cv)               # B = t23
        tta(A, A, Bv)                 # A = t0123
        term0(4, Bv)
        term0(5, Cv)
        tta(Bv, Bv, Cv)               # B = t45
        term0(6, Cv)
        tta(Bv, Bv, Cv)               # B = t456
        term0(7, Cv)
        tta(Bv, Bv, Cv)               # B = t4567
        tta(A, A, Bv)
        term0(8, Bv)
        Wv0 = w_pool.tile([P, F], BF16)
        tta(Wv0, A, Bv)
        Wvs = [Wv0, None]

        def fetch_tile1():
            PI = pi_pool.tile([P, NS * NI], BF16, tag="pic")
            nc.sync.dma_start(
                PI[:], APx(pic_d, P * NS * NI, [[NS * NI, P], [1, NS * NI]]))
            quads = []
            for q in range(2):
                PJ = pjq_pool.tile([P, 4 * F], BF16, tag="pjq")
                nc.sync.dma_start(
                    PJ[:], APx(pjq_d, q * P * 4 * F,
                               [[4 * F, P], [1, 4 * F]]))
                quads.append(PJ)
            PS = pjs_pool.tile([P, F], BF16, tag="pjs")
            nc.sync.dma_start(PS[:], APx(pjs_d, 0, [[F, P], [1, F]]))
            return quads, PS, PI

        def einsum_tile1(quads, PS, PI):
            """Mults batch 4 terms/instr, multiplied in place over the
            quad buffers; batched fold adds."""
            for q, PJ in enumerate(quads):
                v = APx(PJ, 0, [[4 * F, P], [F, 4], [NI, K], [1, NI]])
                nc.vector.tensor_tensor(v, pi_bcast(PI, 4 * q, 4), v,
                                        op=mult)
            A1 = a_pool.tile([P, F], BF16, tag="a")
            nc.vector.tensor_tensor(flat(A1), pi_bcast(PI, 8),
                                    flat(PS), op=mult)
            Q0, Q1 = quads
            nc.vector.tensor_tensor(flat(Q0, 0, 4 * F), flat(Q0, 0, 4 * F),
                                    flat(Q1, 0, 4 * F), op=add)
            nc.vector.tensor_tensor(flat(Q0, 0, F2), flat(Q0, 0, F2),
                                    flat(Q0, F2, F2), op=add)
            Wv = w_pool.tile([P, F], BF16)
            nc.vector.tensor_tensor(flat(Wv), flat(Q0, 0, F),
                                    flat(Q0, F, F), op=add)
            nc.vector.tensor_tensor(flat(Wv), flat(Wv), flat(A1), op=add)
            return Wv

        # ---- per-(tile, head-pair) phase, software-pipelined
        pend = None  # (Y2, Rb2, out_offset)

        def emit_norm(p):
            Y2, Rb2, off = p
            O2 = o_pool.tile([P, F2], BF16)
            nc.vector.tensor_tensor(
                APx(O2, 0, [[F2, P], [F, 2], [NI, K], [1, NI]]),
                APx(Y2, 0, [[F2, P], [F, 2], [NI, K], [1, NI]]),
                APx(Rb2, 0, [[2 * NI, P], [NI, 2], [0, K], [1, NI]]),
                op=mult)
            nc.sync.dma_start(
                APx(out_d, off, [[F, P], [P * F, 2], [1, F]]),
                flat(O2, 0, F2))

        def seg2(t, hstride, c0, n):
            return APx(t, c0 * NI, [[t.tensor.shape[1], P],
                                    [hstride, 2], [NI, n], [1, NI]])

        for T in range(NT):
            Wv = Wvs[T]
            for pr in range(2):
                off = (T * HD + 2 * pr) * P * F
                E2 = e_pool.tile([P, F2], BF16)
                nc.sync.dma_start(
                    APx(E2, 0, [[F2, P], [F, 2], [1, F]]),
                    APx(attn_d, off, [[F, P], [P * F, 2], [1, F]]))
                if T == 0 and pr == 0:
                    in1 = fetch_tile1()
                X2 = x_pool.tile([P, F2], BF16)
                nc.scalar.activation(flat(X2, 0, F2), flat(E2, 0, F2),
                                     mybir.ActivationFunctionType.Exp)
                Y2 = y_pool.tile([P, F2], BF16)
                nc.vector.tensor_tensor(
                    APx(Y2, 0, [[F2, P], [F, 2], [1, F]]),
                    APx(X2, 0, [[F2, P], [F, 2], [1, F]]),
                    APx(Wv, 0, [[F, P], [0, 2], [1, F]]), op=mult)
                if pend is not None:
                    emit_norm(pend)
                    pend = None
                # fold tree 48->24->12->6->3 pairs, then stray cols
                S = f_pool.tile([P, 2 * 24 * NI], BF16, tag="fold")
                D2 = d_pool.tile([P, 2 * NI], F32, tag="d")
                nc.vector.tensor_tensor(seg2(S, 24 * NI, 0, 24),
                                        seg2(Y2, F, 0, 24),
                                        seg2(Y2, F, 24, 24), op=add)
                nc.vector.tensor_tensor(seg2(S, 24 * NI, 0, 12),
                                        seg2(S, 24 * NI, 0, 12),
                                        seg2(S, 24 * NI, 12, 12), op=add)
                nc.vector.tensor_tensor(seg2(S, 24 * NI, 0, 6),
                                        seg2(S, 24 * NI, 0, 6),
                                        seg2(S, 24 * NI, 6, 6), op=add)
                nc.vector.tensor_tensor(seg2(S, 24 * NI, 0, 3),
                                        seg2(S, 24 * NI, 0, 3),
                                        seg2(S, 24 * NI, 3, 3), op=add)
                # live: S[0], S[1], S[2] and Y2 col 48 (per head)
                nc.vector.tensor_tensor(seg2(S, 24 * NI, 0, 1),
                                        seg2(S, 24 * NI, 0, 1),
                                        seg2(Y2, F, 48, 1), op=add)
                nc.vector.tensor_tensor(seg2(S, 24 * NI, 1, 1),
                                        seg2(S, 24 * NI, 1, 1),
                                        seg2(S, 24 * NI, 2, 1), op=add)
                nc.vector.tensor_tensor(
                    APx(D2, 0, [[2 * NI, P], [NI, 2], [1, NI]]),
                    APx(S, 0, [[2 * 24 * NI, P], [24 * NI, 2], [1, NI]]),
                    APx(S, NI, [[2 * 24 * NI, P], [24 * NI, 2], [1, NI]]),
                    op=add)
                R2 = r_pool.tile([P, 2 * NI], F32, tag="r")
                nc.vector.reciprocal(R2[:], D2[:])
                Rb2 = rb_pool.tile([P, 2 * NI], BF16, tag="rb")
                nc.scalar.copy(Rb2[:], R2[:])
                pend = (Y2, Rb2, off)
            if T == 0:
                emit_norm(pend)
                pend = None
                Wvs[1] = einsum_tile1(*in1)
        emit_norm(pend)

    nc.compile()
    return nc


def shard_inputs(attn, sims):
    """Full inputs -> per-core in_maps (list of 8 dicts)."""
    import ml_dtypes
    attn = np.ascontiguousarray(attn, dtype=np.float32)
    sims = np.ascontiguousarray(sims, dtype=np.float32)
    in_maps = []
    rh = np.arange(14)
    dhw = np.arange(3) - 1
    for c in range(N_CORES):
        b, j = divmod(c, 4)
        # attn: (hd, 64, 256, 49) -> [T, hd, p=(hbl,wb), k, i=(ih,iw)]
        a = attn[b, :, BAND * j:BAND * j + BAND]
        a = a.reshape(HD, NT, 4, 8, 32, 8, K)        # hd T hbl ih wb iw k
        a = a.transpose(1, 0, 2, 4, 6, 3, 5)         # T hd hbl wb k ih iw
        attn_shard = np.ascontiguousarray(
            a.reshape(NT * HD, P, F).astype(ml_dtypes.bfloat16))

        # superpixel-factor gather over the 14x14 region per block
        sb = sims[b]                                  # (256,256,32,32)
        gbr = (8 * j + 4 * np.arange(NT)[:, None]
               + np.arange(4)[None, :])               # (T, hbl) block rows
        gh = np.clip(gbr[:, :, None] * 8 + rh[None, None, :] - 3,
                     0, H - 1)                        # (T, hbl, 14)
        gw = np.clip(np.arange(32)[:, None] * 8 + rh[None, :] - 3,
                     0, W - 1)                        # (wb, 14)
        sph = gbr[:, :, None] + dhw[None, None, :]    # (T, hbl, 3)
        spw = np.arange(32)[:, None] + dhw[None, :]   # (wb, 3)
        vh = (sph >= 0) & (sph < SH)
        vw = (spw >= 0) & (spw < SW)
        sphc = np.clip(sph, 0, SH - 1)
        spwc = np.clip(spw, 0, SW - 1)
        # g: (T, hbl, wb, dh, dw, rh14, rw14)
        g = sb[gh[:, :, None, None, None, :, None],
               gw[None, None, :, None, None, None, :],
               sphc[:, :, None, :, None, None, None],
               spwc[None, None, :, None, :, None, None]]
        g *= (vh[:, :, None, :, None, None, None]
              & vw[None, None, :, None, :, None, None])
        # PiC[T, p, s, i]: center 8x8 of each region
        pic = np.ascontiguousarray(
            g[..., 3:11, 3:11].reshape(NT, P, NS * NI)
        ).astype(ml_dtypes.bfloat16)
        # PjX[T, p, s, k, i]: 7x7 sliding windows, k-major
        wnd = np.lib.stride_tricks.sliding_window_view(g, (7, 7), axis=(5, 6))
        # wnd: (T, hbl, wb, dh, dw, ih8, iw8, kh7, kw7)
        pjx = wnd.transpose(0, 1, 2, 3, 4, 7, 8, 5, 6)  # ... kh kw ih iw
        pjx = pjx.reshape(NT, P, NS, F)
        pjt = np.ascontiguousarray(
            pjx[0].transpose(1, 0, 2)).astype(ml_dtypes.bfloat16)
        pjq = np.ascontiguousarray(
            pjx[1, :, :8].reshape(P, 2, 4 * F).transpose(1, 0, 2)
        ).astype(ml_dtypes.bfloat16)
        pjs = np.ascontiguousarray(pjx[1, :, 8]).astype(ml_dtypes.bfloat16)
        in_maps.append({"attn": attn_shard, "pjt": pjt, "pjq": pjq,
                        "pjs": pjs, "pic": pic})
    return in_maps


def unshard_output(results):
    out = np.empty((B, HD, H, W, K), dtype=np.float32)
    for c in range(N_CORES):
        b, j = divmod(c, 4)
        o = results[c]["out"].astype(np.float32)
        o = o.reshape(NT, HD, 4, 32, K, 8, 8)        # T hd hbl wb k ih iw
        o = o.transpose(1, 0, 2, 5, 3, 6, 4)         # hd T hbl ih wb iw k
        out[b, :, BAND * j:BAND * j + BAND] = o.reshape(HD, BAND, W, K)
    return out


_NC_CACHE = {}


def kernel(attn, sims):
    from concourse.bass_utils import run_bass_kernel_spmd
    if "nc" not in _NC_CACHE:
        _NC_CACHE["nc"] = build_graph()
    nc = _NC_CACHE["nc"]
    in_maps = shard_inputs(attn, sims)
    res = run_bass_kernel_spmd(nc, in_maps, core_ids=list(range(N_CORES)))
    return unshard_output(res.results)


# revision 11
# speedup vs baseline: 2.3375x; 1.0101x over previous
"""Trainium2 Bass kernel for nn_AttnReweight (superpixel-reweighted attention).

Math (per batch b, head hd, pixel (h,w), key k in a 7x7 window):
    w[h,w,k] = sum_{s in 3x3 superpixel nbhd} Pi[h,w,s] * Pj[s,h,w,k]
    out = (w * exp(attn)) / sum_k (w * exp(attn))
(The reference's max-shift cancels in the ratio; attn ~ N(0,1) so exp() is
safe without it. eps=1e-15 is negligible vs the denominator ~O(10).)

Sharding: 8 cores = 2 batches x 4 row-bands of 64 rows. Per-core, all
host-prepped gathers, all bf16, k-major free layout [k*64 + i] (k = key
offset in the 7x7 window, i = pixel in the 8x8 block; p = 128 blocks of a
32-row tile half).  k-major makes every device op a packed unit-stride
DVE op, including the per-pixel normalize broadcast (stride-0 over k,
innermost i) and the k-reduction fold tree (packed-64 segments).

  - attn shard: [T, hd, p, k*64+i]
  - PjX: the superpixel factor at the key pixel, pre-expanded per term
    (pure gather): PjX[T, p, s, k*64+i] = sims[b, hj, wj, sph(s), spw(s)],
    zero outside the 32x32 superpixel grid.  Tile 0's nine terms stream
    as single-term chunks (first multiply starts ~3us in); tile 1's
    first eight terms come as two 4-term quads (mults batch 4 terms per
    instruction, multiplied in place over the quad buffer) loaded during
    tile 0's head phase.
  - PiC: the query-pixel factor, compact: PiC[T, p, s*64+i].

Everything computes on DVE except exp / bf16 reciprocal casts (ACT):
GPSIMD work poisons DVE throughput via SBUF contention, and the ISA's
3-free-dim AP limit plus ~225ns/instr overhead favor flat batched ops.
Heads are processed in PAIRS (one instruction covers both heads' grids)
to halve instruction count.  Output unshard + fp32 cast on host.
"""

import sys

sys.path.insert(0, "/opt/trn_rl_repo")

import numpy as np

import concourse.bass as bass
import concourse.tile as tile
from concourse import bacc, mybir
from contextlib import ExitStack

F32 = mybir.dt.float32
BF16 = mybir.dt.bfloat16

# problem geometry (hardcoded per the harness contract)
B, HD, H, W, K = 2, 4, 256, 256, 49
SH = SW = 32
N_CORES = 8
BAND = 64                 # pixel rows per core
NT = 2                    # tile halves (32 rows each) per core
P = 128                   # blocks per tile: 4 block-rows x 32 block-cols
NI = 64                   # pixels per block (8x8)
F = K * NI                # 3136 free elements per (tile, head)
F2 = 2 * F
NS = 9

mult, add = mybir.AluOpType.mult, mybir.AluOpType.add


def APx(t, off, dims):
    return bass.AP(t.tensor, off, [list(d) for d in dims])


def build_graph():
    nc = bacc.Bacc("TRN2", target_bir_lowering=False, debug=False,
                   num_devices=N_CORES)
    attn_d = nc.dram_tensor("attn", [NT * HD, P, F], BF16,
                            kind="ExternalInput").ap()
    pjt_d = nc.dram_tensor("pjt", [NS, P, F], BF16,
                           kind="ExternalInput").ap()   # tile 0, per term
    pjq_d = nc.dram_tensor("pjq", [2, P, 4 * F], BF16,
                           kind="ExternalInput").ap()   # tile 1, quads
    pjs_d = nc.dram_tensor("pjs", [P, F], BF16,
                           kind="ExternalInput").ap()   # tile 1, term 8
    pic_d = nc.dram_tensor("pic", [NT, P, NS * NI], BF16,
                           kind="ExternalInput").ap()
    out_d = nc.dram_tensor("out", [NT * HD, P, F], BF16,
                           kind="ExternalOutput").ap()

    with tile.TileContext(nc) as tc, ExitStack() as ctx:
        pjt_pool = ctx.enter_context(tc.tile_pool(name="pjt", bufs=3))
        pjq_pool = ctx.enter_context(tc.tile_pool(name="pjq", bufs=2))
        pjs_pool = ctx.enter_context(tc.tile_pool(name="pjs", bufs=1))
        pi_pool = ctx.enter_context(tc.tile_pool(name="pic", bufs=2))
        w_pool = ctx.enter_context(tc.tile_pool(name="wv", bufs=2))
        a_pool = ctx.enter_context(tc.tile_pool(name="ea", bufs=1))
        b_pool = ctx.enter_context(tc.tile_pool(name="eb", bufs=1))
        c_pool = ctx.enter_context(tc.tile_pool(name="ec", bufs=1))
        e_pool = ctx.enter_context(tc.tile_pool(name="e2", bufs=2))
        x_pool = ctx.enter_context(tc.tile_pool(name="x2", bufs=1))
        y_pool = ctx.enter_context(tc.tile_pool(name="y2", bufs=2))
        f_pool = ctx.enter_context(tc.tile_pool(name="fold", bufs=2))
        d_pool = ctx.enter_context(tc.tile_pool(name="d2", bufs=2))
        r_pool = ctx.enter_context(tc.tile_pool(name="r2", bufs=2))
        rb_pool = ctx.enter_context(tc.tile_pool(name="rb2", bufs=2))
        o_pool = ctx.enter_context(tc.tile_pool(name="o2", bufs=1))

        def flat(t, off=0, n=F):
            return APx(t, off, [[t.tensor.shape[1], P], [1, n]])

        def tta(dst, a, b):
            nc.vector.tensor_tensor(flat(dst), flat(a), flat(b), op=add)

        def pi_bcast(PI, si, ns=1):
            if ns == 1:
                return APx(PI, si * NI, [[NS * NI, P], [0, K], [1, NI]])
            return APx(PI, si * NI, [[NS * NI, P], [NI, ns], [0, K], [1, NI]])

        # ---- tile 0: chunk 0 first (it gates the first multiply),
        # then PiC, then the remaining term chunks
        chunks = []

        def fetch_chunk(si):
            CH = pjt_pool.tile([P, F], BF16, tag="pjt")
            nc.sync.dma_start(
                CH[:], APx(pjt_d, si * P * F, [[F, P], [1, F]]))
            chunks.append(CH)

        fetch_chunk(0)
        PI0 = pi_pool.tile([P, NS * NI], BF16, tag="pic")
        nc.sync.dma_start(
            PI0[:], APx(pic_d, 0, [[NS * NI, P], [1, NS * NI]]))
        for si in range(1, NS):
            fetch_chunk(si)

        def term0(si, dst):
            nc.vector.tensor_tensor(flat(dst), pi_bcast(PI0, si),
                                    flat(chunks[si]), op=mult)

        A = a_pool.tile([P, F], BF16, tag="a")
        Bv = b_pool.tile([P, F], BF16, tag="b")
        Cv = c_pool.tile([P, F], BF16, tag="c")
        term0(0, A)
        term0(1, Bv)
        tta(A, A, Bv)                 # A = t01
        term0(2, Bv)
        term0(3, Cv)
        tta(Bv, Bv, Cv)               # B = t23
        tta(A, A, Bv)                 # A = t0123
        term0(4, Bv)
        term0(5, Cv)
        tta(Bv, Bv, Cv)               # B = t45
        term0(6, Cv)
        tta(Bv, Bv, Cv)               # B = t456
        term0(7, Cv)
        tta(Bv, Bv, Cv)               # B = t4567
        tta(A, A, Bv)
        term0(8, Bv)
        Wv0 = w_pool.tile([P, F], BF16)
        tta(Wv0, A, Bv)
        Wvs = [Wv0, None]

        def fetch_tile1():
            PI = pi_pool.tile([P, NS * NI], BF16, tag="pic")
            nc.sync.dma_start(
                PI[:], APx(pic_d, P * NS * NI, [[NS * NI, P], [1, NS * NI]]))
            quads = []
            for q in range(2):
                PJ = pjq_pool.tile([P, 4 * F], BF16, tag="pjq")
                nc.sync.dma_start(
                    PJ[:], APx(pjq_d, q * P * 4 * F,
                               [[4 * F, P], [1, 4 * F]]))
                quads.append(PJ)
            PS = pjs_pool.tile([P, F], BF16, tag="pjs")
            nc.sync.dma_start(PS[:], APx(pjs_d, 0, [[F, P], [1, F]]))
            return quads, PS, PI

        def einsum_tile1(quads, PS, PI):
            """Mults batch 4 terms/instr, multiplied in place over the
            quad buffers; batched fold adds."""
            for q, PJ in enumerate(quads):
                v = APx(PJ, 0, [[4 * F, P], [F, 4], [NI, K], [1, NI]])
                nc.vector.tensor_tensor(v, pi_bcast(PI, 4 * q, 4), v,
                                        op=mult)
            A1 = a_pool.tile([P, F], BF16, tag="a")
            nc.vector.tensor_tensor(flat(A1), pi_bcast(PI, 8),
                                    flat(PS), op=mult)
            Q0, Q1 = quads
            nc.vector.tensor_tensor(flat(Q0, 0, 4 * F), flat(Q0, 0, 4 * F),
                                    flat(Q1, 0, 4 * F), op=add)
            nc.vector.tensor_tensor(flat(Q0, 0, F2), flat(Q0, 0, F2),
                                    flat(Q0, F2, F2), op=add)
            Wv = w_pool.tile([P, F], BF16)
            nc.vector.tensor_tensor(flat(Wv), flat(Q0, 0, F),
                                    flat(Q0, F, F), op=add)
            nc.vector.tensor_tensor(flat(Wv), flat(Wv), flat(A1), op=add)
            return Wv

        # ---- per-(tile, head-pair) phase, software-pipelined
        pend = None  # (Y2, Rb2, out_offset)

        def emit_norm(p):
            Y2, Rb2, off = p
            O2 = o_pool.tile([P, F2], BF16)
            nc.vector.tensor_tensor(
                APx(O2, 0, [[F2, P], [F, 2], [NI, K], [1, NI]]),
                APx(Y2, 0, [[F2, P], [F, 2], [NI, K], [1, NI]]),
                APx(Rb2, 0, [[2 * NI, P], [NI, 2], [0, K], [1, NI]]),
                op=mult)
            nc.sync.dma_start(
                APx(out_d, off, [[F, P], [P * F, 2], [1, F]]),
                flat(O2, 0, F2))

        def seg2(t, hstride, c0, n):
            return APx(t, c0 * NI, [[t.tensor.shape[1], P],
                                    [hstride, 2], [NI, n], [1, NI]])

        for T in range(NT):
            Wv = Wvs[T]
            for pr in range(2):
                off = (T * HD + 2 * pr) * P * F
                E2 = e_pool.tile([P, F2], BF16)
                nc.sync.dma_start(
                    APx(E2, 0, [[F2, P], [F, 2], [1, F]]),
                    APx(attn_d, off, [[F, P], [P * F, 2], [1, F]]))
                if T == 0 and pr == 0:
                    in1 = fetch_tile1()
                X2 = x_pool.tile([P, F2], BF16)
                nc.scalar.activation(flat(X2, 0, F2), flat(E2, 0, F2),
                                     mybir.ActivationFunctionType.Exp)
                Y2 = y_pool.tile([P, F2], BF16)
                nc.vector.tensor_tensor(
                    APx(Y2, 0, [[F2, P], [F, 2], [1, F]]),
                    APx(X2, 0, [[F2, P], [F, 2], [1, F]]),
                    APx(Wv, 0, [[F, P], [0, 2], [1, F]]), op=mult)
                if pend is not None:
                    emit_norm(pend)
                    pend = None
                # fold tree 48->24->12->6->3 pairs, then stray cols
                S = f_pool.tile([P, 2 * 24 * NI], BF16, tag="fold")
                D2 = d_pool.tile([P, 2 * NI], F32, tag="d")
                nc.vector.tensor_tensor(seg2(S, 24 * NI, 0, 24),
                                        seg2(Y2, F, 0, 24),
                                        seg2(Y2, F, 24, 24), op=add)
                nc.vector.tensor_tensor(seg2(S, 24 * NI, 0, 12),
                                        seg2(S, 24 * NI, 0, 12),
                                        seg2(S, 24 * NI, 12, 12), op=add)
                nc.vector.tensor_tensor(seg2(S, 24 * NI, 0, 6),
                                        seg2(S, 24 * NI, 0, 6),
                                        seg2(S, 24 * NI, 6, 6), op=add)
                nc.vector.tensor_tensor(seg2(S, 24 * NI, 0, 3),
                                        seg2(S, 24 * NI, 0, 3),
                                        seg2(S, 24 * NI, 3, 3), op=add)
                # live: S[0], S[1], S[2] and Y2 col 48 (per head)
                nc.vector.tensor_tensor(seg2(S, 24 * NI, 0, 1),
                                        seg2(S, 24 * NI, 0, 1),
                                        seg2(Y2, F, 48, 1), op=add)
                nc.vector.tensor_tensor(seg2(S, 24 * NI, 1, 1),
                                        seg2(S, 24 * NI, 1, 1),
                                        seg2(S, 24 * NI, 2, 1), op=add)
                nc.vector.tensor_tensor(
                    APx(D2, 0, [[2 * NI, P], [NI, 2], [1, NI]]),
                    APx(S, 0, [[2 * 24 * NI, P], [24 * NI, 2], [1, NI]]),
                    APx(S, NI, [[2 * 24 * NI, P], [24 * NI, 2], [1, NI]]),
                    op=add)
                R2 = r_pool.tile([P, 2 * NI], F32, tag="r")
                nc.vector.reciprocal(R2[:], D2[:])
                Rb2 = rb_pool.tile([P, 2 * NI], BF16, tag="rb")
                nc.vector.tensor_copy(Rb2[:], R2[:])
                pend = (Y2, Rb2, off)
            if T == 0:
                emit_norm(pend)
                pend = None
                Wvs[1] = einsum_tile1(*in1)
        emit_norm(pend)

    nc.compile()
    return nc


def shard_inputs(attn, sims):
    """Full inputs -> per-core in_maps (list of 8 dicts)."""
    import ml_dtypes
    attn = np.ascontiguousarray(attn, dtype=np.float32)
    sims = np.ascontiguousarray(sims, dtype=np.float32)
    in_maps = []
    rh = np.arange(14)
    dhw = np.arange(3) - 1
    for c in range(N_CORES):
        b, j = divmod(c, 4)
        # attn: (hd, 64, 256, 49) -> [T, hd, p=(hbl,wb), k, i=(ih,iw)]
        a = attn[b, :, BAND * j:BAND * j + BAND]
        a = a.reshape(HD, NT, 4, 8, 32, 8, K)        # hd T hbl ih wb iw k
        a = a.transpose(1, 0, 2, 4, 6, 3, 5)         # T hd hbl wb k ih iw
        attn_shard = np.ascontiguousarray(
            a.reshape(NT * HD, P, F).astype(ml_dtypes.bfloat16))

        # superpixel-factor gather over the 14x14 region per block
        sb = sims[b]                                  # (256,256,32,32)
        gbr = (8 * j + 4 * np.arange(NT)[:, None]
               + np.arange(4)[None, :])               # (T, hbl) block rows
        gh = np.clip(gbr[:, :, None] * 8 + rh[None, None, :] - 3,
                     0, H - 1)                        # (T, hbl, 14)
        gw = np.clip(np.arange(32)[:, None] * 8 + rh[None, :] - 3,
                     0, W - 1)                        # (wb, 14)
        sph = gbr[:, :, None] + dhw[None, None, :]    # (T, hbl, 3)
        spw = np.arange(32)[:, None] + dhw[None, :]   # (wb, 3)
        vh = (sph >= 0) & (sph < SH)
        vw = (spw >= 0) & (spw < SW)
        sphc = np.clip(sph, 0, SH - 1)
        spwc = np.clip(spw, 0, SW - 1)
        # g: (T, hbl, wb, dh, dw, rh14, rw14)
        g = sb[gh[:, :, None, None, None, :, None],
               gw[None, None, :, None, None, None, :],
               sphc[:, :, None, :, None, None, None],
               spwc[None, None, :, None, :, None, None]]
        g *= (vh[:, :, None, :, None, None, None]
              & vw[None, None, :, None, :, None, None])
        # PiC[T, p, s, i]: center 8x8 of each region
        pic = np.ascontiguousarray(
            g[..., 3:11, 3:11].reshape(NT, P, NS * NI)
        ).astype(ml_dtypes.bfloat16)
        # PjX[T, p, s, k, i]: 7x7 sliding windows, k-major
        wnd = np.lib.stride_tricks.sliding_window_view(g, (7, 7), axis=(5, 6))
        # wnd: (T, hbl, wb, dh, dw, ih8, iw8, kh7, kw7)
        pjx = wnd.transpose(0, 1, 2, 3, 4, 7, 8, 5, 6)  # ... kh kw ih iw
        pjx = pjx.reshape(NT, P, NS, F)
        pjt = np.ascontiguousarray(
            pjx[0].transpose(1, 0, 2)).astype(ml_dtypes.bfloat16)
        pjq = np.ascontiguousarray(
            pjx[1, :, :8].reshape(P, 2, 4 * F).transpose(1, 0, 2)
        ).astype(ml_dtypes.bfloat16)
        pjs = np.ascontiguousarray(pjx[1, :, 8]).astype(ml_dtypes.bfloat16)
        in_maps.append({"attn": attn_shard, "pjt": pjt, "pjq": pjq,
                        "pjs": pjs, "pic": pic})
    return in_maps


def unshard_output(results):
    out = np.empty((B, HD, H, W, K), dtype=np.float32)
    for c in range(N_CORES):
        b, j = divmod(c, 4)
        o = results[c]["out"].astype(np.float32)
        o = o.reshape(NT, HD, 4, 32, K, 8, 8)        # T hd hbl wb k ih iw
        o = o.transpose(1, 0, 2, 5, 3, 6, 4)         # hd T hbl ih wb iw k
        out[b, :, BAND * j:BAND * j + BAND] = o.reshape(HD, BAND, W, K)
    return out


_NC_CACHE = {}


def kernel(attn, sims):
    from concourse.bass_utils import run_bass_kernel_spmd
    if "nc" not in _NC_CACHE:
        _NC_CACHE["nc"] = build_graph()
    nc = _NC_CACHE["nc"]
    in_maps = shard_inputs(attn, sims)
    res = run_bass_kernel_spmd(nc, in_maps, core_ids=list(range(N_CORES)))
    return unshard_output(res.results)
